# revision 7
# baseline (speedup 1.0000x reference)
"""AttnDecoderRNN single-step on 8 Trainium2 NeuronCores (tensor parallel).

Math (faithful to the reference, including the softmax-over-singleton bug):
    embedded     = embeddings_index[input_id]                  (H,)
    attn_weights = ones(1, S)                                  (softmax of (S,1) over axis -1)
    attn_applied = column-sums of encoder_outputs              (1, H)
    x            = relu([embedded | attn_applied] @ comb_w.T + comb_b)
    h_new        = GRU_step(x, h0)                             (1, H)
    log_probs    = log_softmax(h_new @ out_w.T + out_b)        (1, V)

Sharding (8 cores):
    - encoder column-sum: rows sharded, AllReduce(add)
    - comb / GRU: output-dim sharded (each core owns a 256-wide slice), AllGather
    - vocab projection: out_w rows sharded (tensor parallel over V);
      per-core sum(exp(logits)) scalars AllReduce(add) -> logZ; each core
      writes its own log_probs shard.

All weight matrices are passed pre-transposed (contraction dim outermost) so
TensorE streams them as the moving operand at line rate.
"""

import numpy as np

H = 2048
S = 2048
V = 50257
NC = 8
P = 128
HC = H // NC          # 256  per-core hidden slice
SC = S // NC          # 256  per-core encoder rows
KC = H // P           # 16   contraction chunks of 128
VS = 6400             # per-core padded vocab rows (VS * NC = 51200 >= V)
PAD_B = -30.0         # bias for padded vocab rows: exp(-30) ~ 9e-14
# vocab column super-groups: (start, width). DMA tiles span the full width;
# PSUM accumulators are <=1024 f32 (2 banks) so 2 groups x 2 bufs + the 4-bank
# chain accumulator fit in the 8 PSUM banks.
VG = [(0, 2048), (2048, 2048), (4096, 2048), (6144, 256)]

_CACHE = {}


def _build():
    import concourse.mybir as mybir
    import concourse.tile as tile
    from concourse import bacc

    fp = mybir.dt.float32
    AF = mybir.ActivationFunctionType
    ALU = mybir.AluOpType
    RG = [list(range(NC))]

    nc = bacc.Bacc(
        "TRN2",
        target_bir_lowering=False,
        debug=False,
        enable_asserts=False,
        num_devices=NC,
    )

    enc_sh = nc.dram_tensor("enc_sh", [SC, H], fp, kind="ExternalInput")
    emb = nc.dram_tensor("emb", [H], fp, kind="ExternalInput")
    h0 = nc.dram_tensor("h0", [H], fp, kind="ExternalInput")
    h0_sl = nc.dram_tensor("h0_sl", [HC], fp, kind="ExternalInput")
    comb_wT = nc.dram_tensor("comb_wT", [2 * H, HC], fp, kind="ExternalInput")
    comb_b = nc.dram_tensor("comb_b", [HC], fp, kind="ExternalInput")
    gru_wT = nc.dram_tensor("gru_wT", [H, 6 * HC], fp, kind="ExternalInput")
    gru_b = nc.dram_tensor("gru_b", [6 * HC], fp, kind="ExternalInput")
    out_wT = nc.dram_tensor("out_wT", [H, VS], fp, kind="ExternalInput")
    out_b = nc.dram_tensor("out_b", [VS], fp, kind="ExternalInput")

    out_lp = nc.dram_tensor("out_lp", [VS], fp, kind="ExternalOutput")
    out_h = nc.dram_tensor("out_h", [H], fp, kind="ExternalOutput")

    def row(t):
        return t.ap().rearrange("(a n) -> a n", a=1)

    def kmaj(ap2d):
        # flat (C*P,) -> [P, C] with chunk c in column c
        return ap2d.rearrange("a (c k) -> k (a c)", k=P)

    with tile.TileContext(nc) as tc:
        with (
            tc.tile_pool(name="c1", bufs=1) as c1,
            tc.tile_pool(name="encp", bufs=1) as encp,
            tc.tile_pool(name="cwp", bufs=2) as cwp,
            tc.tile_pool(name="gwp", bufs=4) as gwp,
            tc.tile_pool(name="wvp", bufs=6) as wvp,
            tc.tile_pool(name="exps", bufs=1) as exps,
            tc.tile_pool(name="dram", bufs=1, space="DRAM") as dp,
            tc.tile_pool(name="psc", bufs=1, space="PSUM") as psc,
            tc.tile_pool(name="psv", bufs=2, space="PSUM") as psv,
        ):
            sy = nc.sync
            ve = nc.vector
            se = nc.scalar

            # ---- constants & small loads (issued early) ----
            ones = c1.tile([P, 1], fp, tag="ones")
            ve.memset(ones[:], 1.0)
            cat_km = c1.tile([P, 2 * KC], fp, tag="cat_km")
            sy.dma_start(cat_km[:, 0:KC], kmaj(row(emb)))
            h0_km = c1.tile([P, KC], fp, tag="h0_km")
            sy.dma_start(h0_km[:], kmaj(row(h0)))
            h0sl = c1.tile([1, HC], fp, tag="h0sl")
            sy.dma_start(h0sl[:], row(h0_sl))
            combb = c1.tile([1, HC], fp, tag="combb")
            sy.dma_start(combb[:], row(comb_b))
            grub = c1.tile([1, 6 * HC], fp, tag="grub")
            sy.dma_start(grub[:], row(gru_b))
            outb = c1.tile([1, VS], fp, tag="outb")
            sy.dma_start(outb[:], row(out_b))



            # DRAM bounce buffers for collectives
            cc_attn_i = dp.tile([1, H], fp, tag="cc_attn_i")
            cc_attn_o = dp.tile([1, H], fp, tag="cc_attn_o")
            cc_x_i = dp.tile([1, HC], fp, tag="cc_x_i")
            cc_x_o = dp.tile([1, H], fp, tag="cc_x_o")
            cc_h_i = dp.tile([1, HC], fp, tag="cc_h_i")
            cc_h_o = dp.tile([1, H], fp, tag="cc_h_o")
            cc_s_i = dp.tile([1, 8], fp, tag="cc_s_i")
            cc_s_o = dp.tile([1, 8], fp, tag="cc_s_o")

            # ---- encoder column-sum (rows sharded) + AllReduce ----
            attn_ps = psc.tile([1, H], fp, tag="chainps")
            for t in range(2):
                et = encp.tile([P, H], fp, tag="enc")
                sy.dma_start(et[:], enc_sh.ap()[t * P:(t + 1) * P, :])
                for n4 in range(4):
                    nc.tensor.matmul(
                        attn_ps[0:1, n4 * 512:(n4 + 1) * 512],
                        lhsT=ones[:, 0:1],
                        rhs=et[:, n4 * 512:(n4 + 1) * 512],
                        start=(t == 0),
                        stop=(t == 1),
                    )
            attn_sb = c1.tile([1, H], fp, tag="attn_sb")
            se.copy(attn_sb[:], attn_ps[:])
            sy.dma_start(cc_attn_i[:], attn_sb[:])
            nc.gpsimd.collective_compute(
                "AllReduce", ALU.add, replica_groups=RG,
                ins=[cc_attn_i.opt()], outs=[cc_attn_o.opt()],
            )
            sy.dma_start(cat_km[:, KC:2 * KC], kmaj(cc_attn_o[:]))

            # ---- attn_combine + relu (output-dim sharded) + AllGather ----
            u_ps = psc.tile([1, HC], fp, tag="chainps")
            nc.tensor.matmul(
                u_ps[0:1, :], lhsT=ones[0:1, 0:1], rhs=combb[0:1, :],
                start=True, stop=False,
            )
            for j in range(4):
                ct = cwp.tile([P, 8 * HC], fp, tag="cw")
                sy.dma_start(
                    ct.rearrange("k (c n) -> k c n", c=8),
                    comb_wT.ap().rearrange("(c k) n -> k c n", k=P)[
                        :, 8 * j:8 * (j + 1), :
                    ],
                )
                for cj in range(8):
                    c = 8 * j + cj
                    nc.tensor.matmul(
                        u_ps[0:1, :],
                        lhsT=cat_km[:, c:c + 1],
                        rhs=ct[:, cj * HC:(cj + 1) * HC],
                        start=False,
                        stop=(c == 2 * KC - 1),
                    )
            x_sb = c1.tile([1, HC], fp, tag="x_sb")
            ve.tensor_scalar_max(x_sb[:], u_ps[:], 0.0)
            sy.dma_start(cc_x_i[:], x_sb[:])
            nc.gpsimd.collective_compute(
                "AllGather", ALU.bypass, replica_groups=RG,
                ins=[cc_x_i.opt()], outs=[cc_x_o.opt()],
            )
            x_km = c1.tile([P, KC], fp, tag="x_km")
            sy.dma_start(x_km[:], kmaj(cc_x_o[:]))

            # ---- GRU step (output-dim sharded) + AllGather ----
            # PSUM start=True clears accumulate-flags for the WHOLE 512-f32
            # bank, so every matmul region must be bank-aligned. Layout:
            # bank0 [0:512]     = gx_r|gx_z
            # bank1 [512:768]   = gx_n   (cols 768:1024 unused)
            # bank2 [1024:1536] = gh_r|gh_z
            # bank3 [1536:1792] = gh_n   (cols 1792:2048 unused)
            g_ps = psc.tile([1, 2048], fp, tag="chainps")
            # (psum_off, width, gru_wT col offset, lhs)
            segs = [(0, 512, 0, "x"), (512, 256, 512, "x"),
                    (1024, 512, 768, "h"), (1536, 256, 1280, "h")]
            for off, w, woff, _s in segs:
                nc.tensor.matmul(
                    g_ps[0:1, off:off + w], lhsT=ones[0:1, 0:1],
                    rhs=grub[0:1, woff:woff + w], start=True, stop=False,
                )
            for kc in range(KC):
                gt = gwp.tile([P, 6 * HC], fp, tag="gw")
                sy.dma_start(gt[:], gru_wT.ap()[kc * P:(kc + 1) * P, :])
                for off, w, woff, s in segs:
                    lhs = x_km if s == "x" else h0_km
                    nc.tensor.matmul(
                        g_ps[0:1, off:off + w],
                        lhsT=lhs[:, kc:kc + 1],
                        rhs=gt[:, woff:woff + w],
                        start=False,
                        stop=(kc == KC - 1),
                    )
            gh_sb = c1.tile([1, 3 * HC], fp, tag="gh_sb")
            se.copy(gh_sb[0:1, 0:512], g_ps[0:1, 1024:1536])
            se.copy(gh_sb[0:1, 512:768], g_ps[0:1, 1536:1792])
            rz_sb = c1.tile([1, 2 * HC], fp, tag="rz_sb")
            ve.tensor_add(rz_sb[:], g_ps[0:1, 0:2 * HC], gh_sb[0:1, 0:2 * HC])
            rzs = c1.tile([1, 2 * HC], fp, tag="rzs")
            se.activation(rzs[:], rz_sb[:], AF.Sigmoid)
            t1 = c1.tile([1, HC], fp, tag="t1")
            ve.tensor_mul(t1[:], rzs[0:1, 0:HC], gh_sb[0:1, 2 * HC:3 * HC])
            t2 = c1.tile([1, HC], fp, tag="t2")
            ve.tensor_add(t2[:], g_ps[0:1, 2 * HC:3 * HC], t1[:])
            n_sb = c1.tile([1, HC], fp, tag="n_sb")
            se.activation(n_sb[:], t2[:], AF.Tanh)
            t3 = c1.tile([1, HC], fp, tag="t3")
            ve.tensor_sub(t3[:], h0sl[:], n_sb[:])
            t4 = c1.tile([1, HC], fp, tag="t4")
            ve.tensor_mul(t4[:], rzs[0:1, HC:2 * HC], t3[:])
            hn_sb = c1.tile([1, HC], fp, tag="hn_sb")
            ve.tensor_add(hn_sb[:], n_sb[:], t4[:])
            sy.dma_start(cc_h_i[:], hn_sb[:])
            nc.gpsimd.collective_compute(
                "AllGather", ALU.bypass, replica_groups=RG,
                ins=[cc_h_i.opt()], outs=[cc_h_o.opt()],
            )
            hN_km = c1.tile([P, KC], fp, tag="hN_km")
            sy.dma_start(hN_km[:], kmaj(cc_h_o[:]))
            sy.dma_start(row(out_h), cc_h_o[:])

            # ---- vocab projection (rows sharded) + fused sum(exp) ----
            logits_sb = c1.tile([1, VS], fp, tag="logits_sb")
            sg = c1.tile([1, 8], fp, tag="sg")
            ve.memset(sg[:], 0.0)
            gi = 0
            for g0, gw in VG:
                if gw > 1024:
                    subs = [(g0, 1024), (g0 + 1024, gw - 1024)]
                else:
                    subs = [(g0, gw)]
                ps_list = [
                    psv.tile([1, sw], fp, tag="vps", name=f"vps_{s0}")
                    for (s0, sw) in subs
                ]
                for (s0, sw), pst in zip(subs, ps_list):
                    for ns0 in range(0, sw, 512):
                        w5 = min(512, sw - ns0)
                        nc.tensor.matmul(
                            pst[0:1, ns0:ns0 + w5],
                            lhsT=ones[0:1, 0:1],
                            rhs=outb[0:1, s0 + ns0:s0 + ns0 + w5],
                            start=True, stop=False,
                        )
                for kc in range(KC):
                    wt = wvp.tile([P, gw], fp, tag="wv")
                    sy.dma_start(
                        wt[:], out_wT.ap()[kc * P:(kc + 1) * P, g0:g0 + gw]
                    )
                    for (s0, sw), pst in zip(subs, ps_list):
                        for ns0 in range(0, sw, 512):
                            w5 = min(512, sw - ns0)
                            nc.tensor.matmul(
                                pst[0:1, ns0:ns0 + w5],
                                lhsT=hN_km[:, kc:kc + 1],
                                rhs=wt[:, (s0 - g0) + ns0:(s0 - g0) + ns0 + w5],
                                start=False,
                                stop=(kc == KC - 1),
                            )
                for (s0, sw), pst in zip(subs, ps_list):
                    se.copy(logits_sb[0:1, s0:s0 + sw], pst[0:1, :])
                    ex = exps.tile([1, 1024], fp, tag="ex")
                    se.activation(
                        ex[0:1, 0:sw], pst[0:1, :], AF.Exp,
                        accum_out=sg[0:1, gi:gi + 1],
                    )
                    gi += 1

            # ---- logZ (AllReduce of local sum-exp) + subtract + store ----
            sloc = c1.tile([1, 8], fp, tag="sloc")
            ve.memset(sloc[:], 0.0)
            ve.tensor_reduce(
                sloc[0:1, 0:1], sg[0:1, 0:gi], axis=mybir.AxisListType.X,
                op=ALU.add,
            )
            sy.dma_start(cc_s_i[:], sloc[:])
            nc.gpsimd.collective_compute(
                "AllReduce", ALU.add, replica_groups=RG,
                ins=[cc_s_i.opt()], outs=[cc_s_o.opt()],
            )
            s_sb = c1.tile([1, 8], fp, tag="s_sb")
            sy.dma_start(s_sb[:], cc_s_o[:])
            logz = c1.tile([1, 1], fp, tag="logz")
            se.activation(logz[:], s_sb[0:1, 0:1], AF.Ln)
            lp_sb = c1.tile([1, VS], fp, tag="lp_sb")
            ve.tensor_scalar_sub(lp_sb[:], logits_sb[:], logz[0:1, 0:1])
            sy.dma_start(row(out_lp), lp_sb[:])

    nc.compile()
    return nc


def _get_compiled():
    if "nc" not in _CACHE:
        _CACHE["nc"] = _build()
    return _CACHE["nc"]


def _prep(inputs):
    f = np.float32
    input_id = int(np.asarray(inputs["input_id"]))
    hidden = np.ascontiguousarray(np.asarray(inputs["hidden"], f).reshape(H))
    enc = np.ascontiguousarray(np.asarray(inputs["encoder_outputs"], f))
    embeddings = np.asarray(inputs["embeddings_index"], f)
    comb_w = np.asarray(inputs["comb_w"], f)
    comb_b = np.asarray(inputs["comb_b"], f)
    w_ih = np.asarray(inputs["w_ih"], f)
    w_hh = np.asarray(inputs["w_hh"], f)
    b_ih = np.asarray(inputs["b_ih"], f)
    b_hh = np.asarray(inputs["b_hh"], f)
    out_w = np.asarray(inputs["out_w"], f)
    out_bv = np.asarray(inputs["out_b"], f)

    emb_row = np.ascontiguousarray(embeddings[input_id])
    maps = []
    for c in range(NC):
        lo, hi = c * HC, (c + 1) * HC
        gsel = np.concatenate(
            [w_ih[lo:hi], w_ih[H + lo:H + hi], w_ih[2 * H + lo:2 * H + hi],
             w_hh[lo:hi], w_hh[H + lo:H + hi], w_hh[2 * H + lo:2 * H + hi]],
            axis=0,
        )
        gb = np.concatenate(
            [b_ih[lo:hi], b_ih[H + lo:H + hi], b_ih[2 * H + lo:2 * H + hi],
             b_hh[lo:hi], b_hh[H + lo:H + hi], b_hh[2 * H + lo:2 * H + hi]],
        )
        v0 = c * VS
        nrows = min(VS, max(0, V - v0))
        wsh = np.zeros((VS, H), f)
        wsh[:nrows] = out_w[v0:v0 + nrows]
        bsh = np.full((VS,), PAD_B, f)
        bsh[:nrows] = out_bv[v0:v0 + nrows]
        maps.append({
            "enc_sh": np.ascontiguousarray(enc[c * SC:(c + 1) * SC]),
            "emb": emb_row,
            "h0": hidden,
            "h0_sl": np.ascontiguousarray(hidden[lo:hi]),
            "comb_wT": np.ascontiguousarray(comb_w[lo:hi, :].T),
            "comb_b": np.ascontiguousarray(comb_b[lo:hi]),
            "gru_wT": np.ascontiguousarray(gsel.T),
            "gru_b": np.ascontiguousarray(gb),
            "out_wT": np.ascontiguousarray(wsh.T),
            "out_b": bsh,
        })
    return maps


def _assemble(results):
    lp = np.concatenate([results[c]["out_lp"] for c in range(NC)])[:V]
    log_probs = np.ascontiguousarray(lp.reshape(1, V))
    h_new = np.ascontiguousarray(results[0]["out_h"].reshape(1, 1, H))
    attn_weights = np.ones((1, S), np.float32)
    return log_probs, h_new, attn_weights


def _run(inputs, trace=False, trace_cores=None):
    import concourse.bass_utils as bass_utils

    nc = _get_compiled()
    maps = _prep(inputs)
    res = bass_utils.run_bass_kernel_spmd(
        nc, maps, core_ids=list(range(NC)), trace=trace, trace_cores=trace_cores,
    )
    return res


def kernel(**inputs):
    res = _run(inputs, trace=False)
    return _assemble(res.results)


# revision 22
# speedup vs baseline: 1.4771x; 1.4771x over previous
"""AttnDecoderRNN single-step on 8 Trainium2 NeuronCores (tensor parallel).

Math (faithful to the reference, including the softmax-over-singleton bug):
    embedded     = embeddings_index[input_id]                  (H,)
    attn_weights = ones(1, S)                                  (softmax of (S,1) over axis -1)
    attn_applied = column-sums of encoder_outputs              (1, H)
    x            = relu([embedded | attn_applied] @ comb_w.T + comb_b)
    h_new        = GRU_step(x, h0)                             (1, H)
    log_probs    = log_softmax(h_new @ out_w.T + out_b)        (1, V)

Sharding (8 cores):
    - encoder column-sum: rows sharded, AllReduce(add)
    - comb / GRU: output-dim sharded (each core owns a 256-wide slice), AllGather
    - vocab projection: out_w rows sharded (tensor parallel over V);
      per-core sum(exp(logits)) scalars AllReduce(add) -> logZ; each core
      writes its own log_probs shard.

All weight matrices are passed pre-transposed (contraction dim outermost) so
TensorE streams them as the moving operand at line rate.
"""

import numpy as np

H = 2048
S = 2048
V = 50257
NC = 8
P = 128
HC = H // NC          # 256  per-core hidden slice
SC = S // NC          # 256  per-core encoder rows
KC = H // P           # 16   contraction chunks of 128
VS = 6400             # per-core padded vocab rows (VS * NC = 51200 >= V)
PAD_B = -30.0         # bias for padded vocab rows: exp(-30) ~ 9e-14
# vocab column super-groups: (start, width). DMA tiles span the full width;
# PSUM accumulators are <=1024 f32 (2 banks) so 2 groups x 2 bufs + the 4-bank
# chain accumulator fit in the 8 PSUM banks.
VG = [(0, 2048), (2048, 2048), (4096, 2048), (6144, 256)]

_CACHE = {}


def _build():
    import concourse.mybir as mybir
    import concourse.tile as tile
    from concourse import bacc

    fp = mybir.dt.float32
    bf = mybir.dt.bfloat16
    AF = mybir.ActivationFunctionType
    ALU = mybir.AluOpType
    RG = [list(range(NC))]

    # fp22 TensorE path: 1 cycle/row instead of fp32's 4 (2 half-speed
    # passes). All PE-feeding tensors are declared float32r end-to-end
    # (same bits as f32 in DRAM; the PE truncates mantissas on read).
    fr = mybir.dt.float32r

    nc = bacc.Bacc(
        "TRN2",
        target_bir_lowering=False,
        debug=False,
        enable_asserts=False,
        num_devices=NC,
    )

    ones_in = nc.dram_tensor("ones_in", [P], fr, kind="ExternalInput")
    enc_sh = nc.dram_tensor("enc_sh", [SC, H], fr, kind="ExternalInput")
    emb = nc.dram_tensor("emb", [H], fr, kind="ExternalInput")
    h0 = nc.dram_tensor("h0", [H], fr, kind="ExternalInput")
    h0_sl = nc.dram_tensor("h0_sl", [HC], fp, kind="ExternalInput")
    comb_wT = nc.dram_tensor("comb_wT", [2 * H, HC], fr, kind="ExternalInput")
    comb_b = nc.dram_tensor("comb_b", [HC], fr, kind="ExternalInput")
    gru_wT = nc.dram_tensor("gru_wT", [H, 6 * HC], fr, kind="ExternalInput")
    gru_b = nc.dram_tensor("gru_b", [6 * HC], fr, kind="ExternalInput")
    out_wT = nc.dram_tensor("out_wT", [H, VS], bf, kind="ExternalInput")
    out_b = nc.dram_tensor("out_b", [VS], fr, kind="ExternalInput")

    out_lp = nc.dram_tensor("out_lp", [VS], fp, kind="ExternalOutput")
    out_h = nc.dram_tensor("out_h", [H], fp, kind="ExternalOutput")

    def row(t):
        return t.ap().rearrange("(a n) -> a n", a=1)

    def kmaj(ap2d):
        # flat (C*P,) -> [P, C] with chunk c in column c
        return ap2d.rearrange("a (c k) -> k (a c)", k=P)

    with tile.TileContext(nc) as tc:
        with (
            tc.tile_pool(name="c1", bufs=1) as c1,
            tc.tile_pool(name="encp", bufs=1) as encp,
            tc.tile_pool(name="cwp", bufs=2) as cwp,
            tc.tile_pool(name="gwp", bufs=4) as gwp,
            tc.tile_pool(name="wvp", bufs=10) as wvp,
            tc.tile_pool(name="exps", bufs=1) as exps,
            tc.tile_pool(name="dram", bufs=1, space="DRAM") as dp,
            tc.tile_pool(name="psc", bufs=1, space="PSUM") as psc,
            tc.tile_pool(name="psv", bufs=2, space="PSUM") as psv,
        ):
            sy = nc.sync
            ve = nc.vector
            se = nc.scalar

            # ---- constants & small loads (issued early) ----
            ones = c1.tile([P, 1], fr, tag="ones")
            sy.dma_start(ones[:], ones_in.ap().rearrange("(k a) -> k a", a=1))
            cat_km = c1.tile([P, 2 * KC], fr, tag="cat_km")
            sy.dma_start(cat_km[:, 0:KC], kmaj(row(emb)))
            h0_km = c1.tile([P, KC], fr, tag="h0_km")
            sy.dma_start(h0_km[:], kmaj(row(h0)))
            h0sl = c1.tile([1, HC], fp, tag="h0sl")
            sy.dma_start(h0sl[:], row(h0_sl))
            combb = c1.tile([1, HC], fr, tag="combb")
            sy.dma_start(combb[:], row(comb_b))
            grub = c1.tile([1, 6 * HC], fr, tag="grub")
            sy.dma_start(grub[:], row(gru_b))
            outb = c1.tile([1, VS], fr, tag="outb")
            sy.dma_start(outb[:], row(out_b))



            # DRAM bounce buffers for collectives
            cc_attn_i = dp.tile([1, H], fp, tag="cc_attn_i")
            cc_attn_o = dp.tile([1, H], fp, tag="cc_attn_o")
            cc_x_i = dp.tile([1, HC], fp, tag="cc_x_i")
            cc_x_o = dp.tile([1, H], fp, tag="cc_x_o")
            cc_h_i = dp.tile([1, HC], fp, tag="cc_h_i")
            cc_h_o = dp.tile([1, H], fp, tag="cc_h_o")
            cc_s_i = dp.tile([1, 8], fp, tag="cc_s_i")
            cc_s_o = dp.tile([1, 8], fp, tag="cc_s_o")

            # ---- encoder column-sum (rows sharded) + AllReduce ----
            attn_ps = psc.tile([1, H], fp, tag="chainps")
            for t in range(2):
                et = encp.tile([P, H], fr, tag="enc")
                sy.dma_start(et[:], enc_sh.ap()[t * P:(t + 1) * P, :])
                for n4 in range(4):
                    nc.tensor.matmul(
                        attn_ps[0:1, n4 * 512:(n4 + 1) * 512],
                        lhsT=ones[:, 0:1],
                        rhs=et[:, n4 * 512:(n4 + 1) * 512],
                        start=(t == 0),
                        stop=(t == 1),
                    )
            attn_sb = c1.tile([1, H], fp, tag="attn_sb")
            se.copy(attn_sb[:], attn_ps[:])
            sy.dma_start(cc_attn_i[:], attn_sb[:])
            nc.gpsimd.collective_compute(
                "AllReduce", ALU.add, replica_groups=RG,
                ins=[cc_attn_i.opt()], outs=[cc_attn_o.opt()],
            )
            sy.dma_start(cat_km[:, KC:2 * KC], kmaj(cc_attn_o[:]).bitcast(fr))

            # ---- attn_combine + relu (output-dim sharded) + AllGather ----
            u_ps = psc.tile([1, HC], fp, tag="chainps")
            nc.tensor.matmul(
                u_ps[0:1, :], lhsT=ones[0:1, 0:1], rhs=combb[0:1, :],
                start=True, stop=False,
            )
            for j in range(4):
                ct = cwp.tile([P, 8 * HC], fr, tag="cw")
                sy.dma_start(
                    ct.rearrange("k (c n) -> k c n", c=8),
                    comb_wT.ap().rearrange("(c k) n -> k c n", k=P)[
                        :, 8 * j:8 * (j + 1), :
                    ],
                )
                for cj in range(8):
                    c = 8 * j + cj
                    nc.tensor.matmul(
                        u_ps[0:1, :],
                        lhsT=cat_km[:, c:c + 1],
                        rhs=ct[:, cj * HC:(cj + 1) * HC],
                        start=False,
                        stop=(c == 2 * KC - 1),
                    )
            x_sb = c1.tile([1, HC], fp, tag="x_sb")
            ve.tensor_scalar_max(x_sb[:], u_ps[:], 0.0)
            sy.dma_start(cc_x_i[:], x_sb[:])
            nc.gpsimd.collective_compute(
                "AllGather", ALU.bypass, replica_groups=RG,
                ins=[cc_x_i.opt()], outs=[cc_x_o.opt()],
            )
            x_km = c1.tile([P, KC], fr, tag="x_km")
            sy.dma_start(x_km[:], kmaj(cc_x_o[:]).bitcast(fr))

            # ---- GRU step (output-dim sharded) + AllGather ----
            # PSUM start=True clears accumulate-flags for the WHOLE 512-f32
            # bank, so every matmul region must be bank-aligned. Layout:
            # bank0 [0:512]     = gx_r|gx_z
            # bank1 [512:768]   = gx_n   (cols 768:1024 unused)
            # bank2 [1024:1536] = gh_r|gh_z
            # bank3 [1536:1792] = gh_n   (cols 1792:2048 unused)
            g_ps = psc.tile([1, 2048], fp, tag="chainps")
            # (psum_off, width, gru_wT col offset, lhs)
            segs = [(0, 512, 0, "x"), (512, 256, 512, "x"),
                    (1024, 512, 768, "h"), (1536, 256, 1280, "h")]
            for off, w, woff, _s in segs:
                nc.tensor.matmul(
                    g_ps[0:1, off:off + w], lhsT=ones[0:1, 0:1],
                    rhs=grub[0:1, woff:woff + w], start=True, stop=False,
                )
            for kc in range(KC):
                gt = gwp.tile([P, 6 * HC], fr, tag="gw")
                sy.dma_start(gt[:], gru_wT.ap()[kc * P:(kc + 1) * P, :])
                for off, w, woff, s in segs:
                    lhs = x_km if s == "x" else h0_km
                    nc.tensor.matmul(
                        g_ps[0:1, off:off + w],
                        lhsT=lhs[:, kc:kc + 1],
                        rhs=gt[:, woff:woff + w],
                        start=False,
                        stop=(kc == KC - 1),
                    )
            gh_sb = c1.tile([1, 3 * HC], fp, tag="gh_sb")
            se.copy(gh_sb[0:1, 0:512], g_ps[0:1, 1024:1536])
            se.copy(gh_sb[0:1, 512:768], g_ps[0:1, 1536:1792])
            rz_sb = c1.tile([1, 2 * HC], fp, tag="rz_sb")
            ve.tensor_add(rz_sb[:], g_ps[0:1, 0:2 * HC], gh_sb[0:1, 0:2 * HC])
            rzs = c1.tile([1, 2 * HC], fp, tag="rzs")
            se.activation(rzs[:], rz_sb[:], AF.Sigmoid)
            t1 = c1.tile([1, HC], fp, tag="t1")
            ve.tensor_mul(t1[:], rzs[0:1, 0:HC], gh_sb[0:1, 2 * HC:3 * HC])
            t2 = c1.tile([1, HC], fp, tag="t2")
            ve.tensor_add(t2[:], g_ps[0:1, 2 * HC:3 * HC], t1[:])
            n_sb = c1.tile([1, HC], fp, tag="n_sb")
            se.activation(n_sb[:], t2[:], AF.Tanh)
            t3 = c1.tile([1, HC], fp, tag="t3")
            ve.tensor_sub(t3[:], h0sl[:], n_sb[:])
            t4 = c1.tile([1, HC], fp, tag="t4")
            ve.tensor_mul(t4[:], rzs[0:1, HC:2 * HC], t3[:])
            hn_sb = c1.tile([1, HC], fp, tag="hn_sb")
            ve.tensor_add(hn_sb[:], n_sb[:], t4[:])
            sy.dma_start(cc_h_i[:], hn_sb[:])
            nc.gpsimd.collective_compute(
                "AllGather", ALU.bypass, replica_groups=RG,
                ins=[cc_h_i.opt()], outs=[cc_h_o.opt()],
            )
            hN_km = c1.tile([P, KC], fp, tag="hN_km")
            sy.dma_start(hN_km[:], kmaj(cc_h_o[:]))
            sy.dma_start(row(out_h), cc_h_o[:])
            hN_bf = c1.tile([P, KC], bf, tag="hN_bf")
            ve.tensor_copy(hN_bf[:], hN_km[:])

            # ---- vocab projection (rows sharded) + fused sum(exp) ----
            logits_sb = c1.tile([1, VS], fp, tag="logits_sb")
            sg = c1.tile([1, 8], fp, tag="sg")
            ve.memset(sg[:], 0.0)
            gi = 0
            for g0, gw in VG:
                if gw > 1024:
                    subs = [(g0, 1024), (g0 + 1024, gw - 1024)]
                else:
                    subs = [(g0, gw)]
                ps_list = [
                    psv.tile([1, sw], fp, tag="vps", name=f"vps_{s0}")
                    for (s0, sw) in subs
                ]
                for (s0, sw), pst in zip(subs, ps_list):
                    for ns0 in range(0, sw, 512):
                        w5 = min(512, sw - ns0)
                        nc.tensor.matmul(
                            pst[0:1, ns0:ns0 + w5],
                            lhsT=ones[0:1, 0:1],
                            rhs=outb[0:1, s0 + ns0:s0 + ns0 + w5],
                            start=True, stop=False,
                        )
                for kc in range(KC):
                    wt = wvp.tile([P, gw], bf, tag="wv")
                    sy.dma_start(
                        wt[:], out_wT.ap()[kc * P:(kc + 1) * P, g0:g0 + gw]
                    )
                    for (s0, sw), pst in zip(subs, ps_list):
                        for ns0 in range(0, sw, 512):
                            w5 = min(512, sw - ns0)
                            nc.tensor.matmul(
                                pst[0:1, ns0:ns0 + w5],
                                lhsT=hN_bf[:, kc:kc + 1],
                                rhs=wt[:, (s0 - g0) + ns0:(s0 - g0) + ns0 + w5],
                                start=False,
                                stop=(kc == KC - 1),
                            )
                for (s0, sw), pst in zip(subs, ps_list):
                    se.copy(logits_sb[0:1, s0:s0 + sw], pst[0:1, :])
                    ex = exps.tile([1, 1024], fp, tag="ex")
                    se.activation(
                        ex[0:1, 0:sw], pst[0:1, :], AF.Exp,
                        accum_out=sg[0:1, gi:gi + 1],
                    )
                    gi += 1

            # ---- logZ (AllReduce of local sum-exp) + subtract + store ----
            sloc = c1.tile([1, 8], fp, tag="sloc")
            ve.memset(sloc[:], 0.0)
            ve.tensor_reduce(
                sloc[0:1, 0:1], sg[0:1, 0:gi], axis=mybir.AxisListType.X,
                op=ALU.add,
            )
            sy.dma_start(cc_s_i[:], sloc[:])
            nc.gpsimd.collective_compute(
                "AllReduce", ALU.add, replica_groups=RG,
                ins=[cc_s_i.opt()], outs=[cc_s_o.opt()],
            )
            s_sb = c1.tile([1, 8], fp, tag="s_sb")
            sy.dma_start(s_sb[:], cc_s_o[:])
            logz = c1.tile([1, 1], fp, tag="logz")
            se.activation(logz[:], s_sb[0:1, 0:1], AF.Ln)
            lp_sb = c1.tile([1, VS], fp, tag="lp_sb")
            ve.tensor_scalar_sub(lp_sb[:], logits_sb[:], logz[0:1, 0:1])
            sy.dma_start(row(out_lp), lp_sb[:])

    nc.compile()
    return nc


def _get_compiled():
    if "nc" not in _CACHE:
        _CACHE["nc"] = _build()
    return _CACHE["nc"]


def _prep(inputs):
    f = np.float32
    input_id = int(np.asarray(inputs["input_id"]))
    hidden = np.ascontiguousarray(np.asarray(inputs["hidden"], f).reshape(H))
    enc = np.ascontiguousarray(np.asarray(inputs["encoder_outputs"], f))
    embeddings = np.asarray(inputs["embeddings_index"], f)
    comb_w = np.asarray(inputs["comb_w"], f)
    comb_b = np.asarray(inputs["comb_b"], f)
    w_ih = np.asarray(inputs["w_ih"], f)
    w_hh = np.asarray(inputs["w_hh"], f)
    b_ih = np.asarray(inputs["b_ih"], f)
    b_hh = np.asarray(inputs["b_hh"], f)
    out_w = np.asarray(inputs["out_w"], f)
    out_bv = np.asarray(inputs["out_b"], f)

    emb_row = np.ascontiguousarray(embeddings[input_id])
    maps = []
    for c in range(NC):
        lo, hi = c * HC, (c + 1) * HC
        gsel = np.concatenate(
            [w_ih[lo:hi], w_ih[H + lo:H + hi], w_ih[2 * H + lo:2 * H + hi],
             w_hh[lo:hi], w_hh[H + lo:H + hi], w_hh[2 * H + lo:2 * H + hi]],
            axis=0,
        )
        gb = np.concatenate(
            [b_ih[lo:hi], b_ih[H + lo:H + hi], b_ih[2 * H + lo:2 * H + hi],
             b_hh[lo:hi], b_hh[H + lo:H + hi], b_hh[2 * H + lo:2 * H + hi]],
        )
        import ml_dtypes

        v0 = c * VS
        nrows = min(VS, max(0, V - v0))
        wsh = np.zeros((VS, H), f)
        wsh[:nrows] = out_w[v0:v0 + nrows]
        wsh = wsh.astype(ml_dtypes.bfloat16)
        bsh = np.full((VS,), PAD_B, f)
        bsh[:nrows] = out_bv[v0:v0 + nrows]
        maps.append({
            "ones_in": np.ones((P,), f),
            "enc_sh": np.ascontiguousarray(enc[c * SC:(c + 1) * SC]),
            "emb": emb_row,
            "h0": hidden,
            "h0_sl": np.ascontiguousarray(hidden[lo:hi]),
            "comb_wT": np.ascontiguousarray(comb_w[lo:hi, :].T),
            "comb_b": np.ascontiguousarray(comb_b[lo:hi]),
            "gru_wT": np.ascontiguousarray(gsel.T),
            "gru_b": np.ascontiguousarray(gb),
            "out_wT": np.ascontiguousarray(wsh.T),
            "out_b": bsh,
        })
    return maps


def _assemble(results):
    lp = np.concatenate([results[c]["out_lp"] for c in range(NC)])[:V]
    log_probs = np.ascontiguousarray(lp.reshape(1, V))
    h_new = np.ascontiguousarray(results[0]["out_h"].reshape(1, 1, H))
    attn_weights = np.ones((1, S), np.float32)
    return log_probs, h_new, attn_weights


def _run(inputs, trace=False, trace_cores=None):
    import concourse.bass_utils as bass_utils

    nc = _get_compiled()
    maps = _prep(inputs)
    res = bass_utils.run_bass_kernel_spmd(
        nc, maps, core_ids=list(range(NC)), trace=trace, trace_cores=trace_cores,
    )
    return res


def kernel(**inputs):
    res = _run(inputs, trace=False)
    return _assemble(res.results)


# revision 24
# speedup vs baseline: 1.5489x; 1.0486x over previous
"""AttnDecoderRNN single-step on 8 Trainium2 NeuronCores (tensor parallel).

Math (faithful to the reference, including the softmax-over-singleton bug):
    embedded     = embeddings_index[input_id]                  (H,)
    attn_weights = ones(1, S)                                  (softmax of (S,1) over axis -1)
    attn_applied = column-sums of encoder_outputs              (1, H)
    x            = relu([embedded | attn_applied] @ comb_w.T + comb_b)
    h_new        = GRU_step(x, h0)                             (1, H)
    log_probs    = log_softmax(h_new @ out_w.T + out_b)        (1, V)

Sharding (8 cores):
    - encoder column-sum: rows sharded, AllReduce(add)
    - comb / GRU: output-dim sharded (each core owns a 256-wide slice), AllGather
    - vocab projection: out_w rows sharded (tensor parallel over V);
      per-core sum(exp(logits)) scalars AllGather -> logZ; each core
      writes its own log_probs shard.

Precision: the small chain runs as float32r (fp22 on TensorE, 1 cyc/row);
the dominant vocab projection streams bf16 weights (exact bf16 products,
fp32 accumulate). Verified ~4e-3 max rel err on h_new, ~6e-4 on log_probs.

All weight matrices are passed pre-transposed (contraction dim outermost) so
TensorE streams them as the moving operand at line rate.
"""

import numpy as np

H = 2048
S = 2048
V = 50257
NC = 8
P = 128
HC = H // NC          # 256  per-core hidden slice
SC = S // NC          # 256  per-core encoder rows
KC = H // P           # 16   contraction chunks of 128
VS = 6400             # per-core padded vocab rows (VS * NC = 51200 >= V)
PAD_B = -30.0         # bias for padded vocab rows: exp(-30) ~ 9e-14
# vocab column super-groups: (start, width). One bf16 DMA tile per
# (super-group, k-chunk); PSUM accumulators are 512-col sub-groups.
VG = [(0, 2048), (2048, 2048), (4096, 2048), (6144, 256)]

_CACHE = {}


def _build():
    import concourse.mybir as mybir
    import concourse.tile as tile
    from concourse import bacc

    fp = mybir.dt.float32
    bf = mybir.dt.bfloat16
    AF = mybir.ActivationFunctionType
    ALU = mybir.AluOpType
    RG = [list(range(NC))]

    # fp22 TensorE path: 1 cycle/row instead of fp32's 4 (2 half-speed
    # passes). All PE-feeding tensors are declared float32r end-to-end
    # (same bits as f32 in DRAM; the PE truncates mantissas on read).
    fr = mybir.dt.float32r

    nc = bacc.Bacc(
        "TRN2",
        target_bir_lowering=False,
        debug=False,
        enable_asserts=False,
        num_devices=NC,
    )

    ones_in = nc.dram_tensor("ones_in", [P], fr, kind="ExternalInput")
    enc_sh = nc.dram_tensor("enc_sh", [SC, H], fr, kind="ExternalInput")
    emb = nc.dram_tensor("emb", [H], fr, kind="ExternalInput")
    h0 = nc.dram_tensor("h0", [H], fr, kind="ExternalInput")
    h0_sl = nc.dram_tensor("h0_sl", [HC], fp, kind="ExternalInput")
    comb_wT = nc.dram_tensor("comb_wT", [2 * H, HC], fr, kind="ExternalInput")
    comb_b = nc.dram_tensor("comb_b", [HC], fr, kind="ExternalInput")
    gru_wT = nc.dram_tensor("gru_wT", [H, 6 * HC], fr, kind="ExternalInput")
    gru_b = nc.dram_tensor("gru_b", [6 * HC], fr, kind="ExternalInput")
    out_wT = nc.dram_tensor("out_wT", [H, VS], bf, kind="ExternalInput")
    out_b = nc.dram_tensor("out_b", [VS], fr, kind="ExternalInput")

    out_lp = nc.dram_tensor("out_lp", [VS], fp, kind="ExternalOutput")
    out_h = nc.dram_tensor("out_h", [H], fp, kind="ExternalOutput")

    def row(t):
        return t.ap().rearrange("(a n) -> a n", a=1)

    def kmaj(ap2d):
        # flat (C*P,) -> [P, C] with chunk c in column c
        return ap2d.rearrange("a (c k) -> k (a c)", k=P)

    with tile.TileContext(nc) as tc:
        with (
            tc.tile_pool(name="c1", bufs=1) as c1,
            tc.tile_pool(name="wvp", bufs=13) as wvp,
            tc.tile_pool(name="encp", bufs=1) as encp,
            tc.tile_pool(name="cwp", bufs=2) as cwp,
            tc.tile_pool(name="gwp", bufs=4) as gwp,
            tc.tile_pool(name="dram", bufs=1, space="DRAM") as dp,
        ):
            sy = nc.sync
            ve = nc.vector
            se = nc.scalar

            # ---- constants & small loads (issued early) ----
            ones = c1.tile([P, 1], fr, tag="ones")
            sy.dma_start(ones[:], ones_in.ap().rearrange("(k a) -> k a", a=1))
            cat_km = c1.tile([P, 2 * KC], fr, tag="cat_km")
            sy.dma_start(cat_km[:, 0:KC], kmaj(row(emb)))
            h0_km = c1.tile([P, KC], fr, tag="h0_km")
            sy.dma_start(h0_km[:], kmaj(row(h0)))
            h0sl = c1.tile([1, HC], fp, tag="h0sl")
            sy.dma_start(h0sl[:], row(h0_sl))
            combb = c1.tile([1, HC], fr, tag="combb")
            sy.dma_start(combb[:], row(comb_b))
            grub = c1.tile([1, 6 * HC], fr, tag="grub")
            sy.dma_start(grub[:], row(gru_b))
            outb = c1.tile([1, VS], fr, tag="outb")
            sy.dma_start(outb[:], row(out_b))

            # Warm the natural_log_exp ACT table set so the tail's Ln does
            # not pay a ~2.7us table switch: Ln(0*x + 1) == 0, discarded.
            warm = c1.tile([1, 1], fp, tag="warm")
            se.activation(warm[:], h0sl[0:1, 0:1], AF.Ln, bias=1.0, scale=0.0)

            # DRAM bounce buffers for collectives
            cc_attn_i = dp.tile([1, H], fp, tag="cc_attn_i")
            cc_attn_o = dp.tile([1, H], fp, tag="cc_attn_o")
            cc_x_i = dp.tile([1, HC], fp, tag="cc_x_i")
            cc_x_o = dp.tile([1, H], fp, tag="cc_x_o")
            cc_h_i = dp.tile([1, HC], fp, tag="cc_h_i")
            cc_h_o = dp.tile([1, H], fp, tag="cc_h_o")
            cc_s_i = dp.tile([1, 8], fp, tag="cc_s_i")
            cc_s_o = dp.tile([1, 8 * NC], fp, tag="cc_s_o")

            with tc.tile_pool(name="psc", bufs=1, space="PSUM") as psc:
                # ---- encoder column-sum (rows sharded) + AllReduce ----
                attn_ps = psc.tile([1, H], fp, tag="chainps")
                for t in range(2):
                    et = encp.tile([P, H], fr, tag="enc")
                    sy.dma_start(et[:], enc_sh.ap()[t * P:(t + 1) * P, :])
                    for n4 in range(4):
                        nc.tensor.matmul(
                            attn_ps[0:1, n4 * 512:(n4 + 1) * 512],
                            lhsT=ones[:, 0:1],
                            rhs=et[:, n4 * 512:(n4 + 1) * 512],
                            start=(t == 0),
                            stop=(t == 1),
                        )
                attn_sb = c1.tile([1, H], fp, tag="attn_sb")
                se.copy(attn_sb[:], attn_ps[:])
                sy.dma_start(cc_attn_i[:], attn_sb[:])
                nc.gpsimd.collective_compute(
                    "AllReduce", ALU.add, replica_groups=RG,
                    ins=[cc_attn_i.opt()], outs=[cc_attn_o.opt()],
                )
                sy.dma_start(
                    cat_km[:, KC:2 * KC], kmaj(cc_attn_o[:]).bitcast(fr)
                )

                # ---- attn_combine + relu (output-dim sharded) + AllGather
                u_ps = psc.tile([1, HC], fp, tag="chainps")
                nc.tensor.matmul(
                    u_ps[0:1, :], lhsT=ones[0:1, 0:1], rhs=combb[0:1, :],
                    start=True, stop=False,
                )
                for j in range(4):
                    ct = cwp.tile([P, 8 * HC], fr, tag="cw")
                    sy.dma_start(
                        ct.rearrange("k (c n) -> k c n", c=8),
                        comb_wT.ap().rearrange("(c k) n -> k c n", k=P)[
                            :, 8 * j:8 * (j + 1), :
                        ],
                    )
                    for cj in range(8):
                        c = 8 * j + cj
                        nc.tensor.matmul(
                            u_ps[0:1, :],
                            lhsT=cat_km[:, c:c + 1],
                            rhs=ct[:, cj * HC:(cj + 1) * HC],
                            start=False,
                            stop=(c == 2 * KC - 1),
                        )
                x_sb = c1.tile([1, HC], fp, tag="x_sb")
                ve.tensor_scalar_max(x_sb[:], u_ps[:], 0.0)
                sy.dma_start(cc_x_i[:], x_sb[:])
                nc.gpsimd.collective_compute(
                    "AllGather", ALU.bypass, replica_groups=RG,
                    ins=[cc_x_i.opt()], outs=[cc_x_o.opt()],
                )
                x_km = c1.tile([P, KC], fr, tag="x_km")
                sy.dma_start(x_km[:], kmaj(cc_x_o[:]).bitcast(fr))

                # ---- GRU step (output-dim sharded) + AllGather ----
                # PSUM start=True clears accumulate-flags for the WHOLE
                # 512-f32 bank, so every matmul region must be bank-aligned:
                # bank0 [0:512]     = gx_r|gx_z
                # bank1 [512:768]   = gx_n   (cols 768:1024 unused)
                # bank2 [1024:1536] = gh_r|gh_z
                # bank3 [1536:1792] = gh_n   (cols 1792:2048 unused)
                g_ps = psc.tile([1, 2048], fp, tag="chainps")
                # (psum_off, width, gru_wT col offset, lhs)
                segs = [(0, 512, 0, "x"), (512, 256, 512, "x"),
                        (1024, 512, 768, "h"), (1536, 256, 1280, "h")]
                for off, w, woff, _s in segs:
                    nc.tensor.matmul(
                        g_ps[0:1, off:off + w], lhsT=ones[0:1, 0:1],
                        rhs=grub[0:1, woff:woff + w], start=True, stop=False,
                    )
                for kc in range(KC):
                    gt = gwp.tile([P, 6 * HC], fr, tag="gw")
                    sy.dma_start(gt[:], gru_wT.ap()[kc * P:(kc + 1) * P, :])
                    for off, w, woff, s in segs:
                        lhs = x_km if s == "x" else h0_km
                        nc.tensor.matmul(
                            g_ps[0:1, off:off + w],
                            lhsT=lhs[:, kc:kc + 1],
                            rhs=gt[:, woff:woff + w],
                            start=False,
                            stop=(kc == KC - 1),
                        )
                gh_sb = c1.tile([1, 3 * HC], fp, tag="gh_sb")
                se.copy(gh_sb[0:1, 0:512], g_ps[0:1, 1024:1536])
                se.copy(gh_sb[0:1, 512:768], g_ps[0:1, 1536:1792])
                rz_sb = c1.tile([1, 2 * HC], fp, tag="rz_sb")
                ve.tensor_add(rz_sb[:], g_ps[0:1, 0:2 * HC], gh_sb[0:1, 0:2 * HC])
                rzs = c1.tile([1, 2 * HC], fp, tag="rzs")
                se.activation(rzs[:], rz_sb[:], AF.Sigmoid)
                t1 = c1.tile([1, HC], fp, tag="t1")
                ve.tensor_mul(t1[:], rzs[0:1, 0:HC], gh_sb[0:1, 2 * HC:3 * HC])
                t2 = c1.tile([1, HC], fp, tag="t2")
                ve.tensor_add(t2[:], g_ps[0:1, 512:768], t1[:])
                n_sb = c1.tile([1, HC], fp, tag="n_sb")
                se.activation(n_sb[:], t2[:], AF.Tanh)
                t3 = c1.tile([1, HC], fp, tag="t3")
                ve.tensor_sub(t3[:], h0sl[:], n_sb[:])
                t4 = c1.tile([1, HC], fp, tag="t4")
                ve.tensor_mul(t4[:], rzs[0:1, HC:2 * HC], t3[:])
                hn_sb = c1.tile([1, HC], fp, tag="hn_sb")
                ve.tensor_add(hn_sb[:], n_sb[:], t4[:])
                sy.dma_start(cc_h_i[:], hn_sb[:])
                nc.gpsimd.collective_compute(
                    "AllGather", ALU.bypass, replica_groups=RG,
                    ins=[cc_h_i.opt()], outs=[cc_h_o.opt()],
                )
                hN_km = c1.tile([P, KC], fp, tag="hN_km")
                sy.dma_start(hN_km[:], kmaj(cc_h_o[:]))
                sy.dma_start(row(out_h), cc_h_o[:])
                hN_bf = c1.tile([P, KC], bf, tag="hN_bf")
                ve.tensor_copy(hN_bf[:], hN_km[:])

            # ---- vocab projection (rows sharded) + fused sum(exp) ----
            # chain PSUM pool is closed: the vocab pool gets all 8 banks so
            # two super-groups can be in flight (matmuls of g+1 overlap the
            # epilogue of g).
            logits_sb = c1.tile([1, VS], fp, tag="logits_sb")
            sg = c1.tile([1, 8], fp, tag="sg")
            gi = 0
            with tc.tile_pool(name="psv", bufs=4, space="PSUM") as psv:
                for g0, gw in VG:
                    if gw > 1024:
                        subs = [(g0, 1024), (g0 + 1024, gw - 1024)]
                    else:
                        subs = [(g0, gw)]
                    ps_list = [
                        psv.tile([1, sw], fp, tag="vps", name=f"vps_{s0}")
                        for (s0, sw) in subs
                    ]
                    for (s0, sw), pst in zip(subs, ps_list):
                        for ns0 in range(0, sw, 512):
                            w5 = min(512, sw - ns0)
                            nc.tensor.matmul(
                                pst[0:1, ns0:ns0 + w5],
                                lhsT=ones[0:1, 0:1],
                                rhs=outb[0:1, s0 + ns0:s0 + ns0 + w5],
                                start=True, stop=False,
                            )
                    for kc in range(KC):
                        wt = wvp.tile([P, gw], bf, tag="wv")
                        sy.dma_start(
                            wt[:], out_wT.ap()[kc * P:(kc + 1) * P, g0:g0 + gw]
                        )
                        for (s0, sw), pst in zip(subs, ps_list):
                            for ns0 in range(0, sw, 512):
                                w5 = min(512, sw - ns0)
                                nc.tensor.matmul(
                                    pst[0:1, ns0:ns0 + w5],
                                    lhsT=hN_bf[:, kc:kc + 1],
                                    rhs=wt[:, (s0 - g0) + ns0:(s0 - g0) + ns0 + w5],
                                    start=False,
                                    stop=(kc == KC - 1),
                                )
                    for (s0, sw), pst in zip(subs, ps_list):
                        ve.tensor_copy(logits_sb[0:1, s0:s0 + sw], pst[0:1, :])
                        ex = c1.tile([1, 1024], fp, tag="ex", bufs=2,
                                     name=f"ex_{s0}")
                        se.activation(
                            ex[0:1, 0:sw], pst[0:1, :], AF.Exp,
                            accum_out=sg[0:1, gi:gi + 1],
                        )
                        gi += 1

            # ---- logZ (AllGather of per-core sum-exp) + subtract + store
            sloc = c1.tile([1, 8], fp, tag="sloc")
            ve.memset(sloc[:], 0.0)
            ve.tensor_reduce(
                sloc[0:1, 0:1], sg[0:1, 0:gi], axis=mybir.AxisListType.X,
                op=ALU.add,
            )
            sy.dma_start(cc_s_i[:], sloc[:])
            nc.gpsimd.collective_compute(
                "AllGather", ALU.bypass, replica_groups=RG,
                ins=[cc_s_i.opt()], outs=[cc_s_o.opt()],
            )
            s_sb = c1.tile([1, 8 * NC], fp, tag="s_sb")
            sy.dma_start(s_sb[:], cc_s_o[:])
            s_tot = c1.tile([1, 1], fp, tag="s_tot")
            ve.tensor_reduce(
                s_tot[0:1, 0:1],
                s_sb[:].rearrange("a (r e) -> a r e", e=8)[:, :, 0],
                axis=mybir.AxisListType.X, op=ALU.add,
            )
            logz = c1.tile([1, 1], fp, tag="logz")
            se.activation(logz[:], s_tot[:], AF.Ln)
            ve.tensor_scalar_sub(logits_sb[:], logits_sb[:], logz[0:1, 0:1])
            sy.dma_start(row(out_lp), logits_sb[:])

    nc.compile()
    return nc


def _get_compiled():
    if "nc" not in _CACHE:
        _CACHE["nc"] = _build()
    return _CACHE["nc"]


def _prep(inputs):
    import ml_dtypes

    f = np.float32
    input_id = int(np.asarray(inputs["input_id"]))
    hidden = np.ascontiguousarray(np.asarray(inputs["hidden"], f).reshape(H))
    enc = np.ascontiguousarray(np.asarray(inputs["encoder_outputs"], f))
    embeddings = np.asarray(inputs["embeddings_index"], f)
    comb_w = np.asarray(inputs["comb_w"], f)
    comb_b = np.asarray(inputs["comb_b"], f)
    w_ih = np.asarray(inputs["w_ih"], f)
    w_hh = np.asarray(inputs["w_hh"], f)
    b_ih = np.asarray(inputs["b_ih"], f)
    b_hh = np.asarray(inputs["b_hh"], f)
    out_w = np.asarray(inputs["out_w"], f)
    out_bv = np.asarray(inputs["out_b"], f)

    emb_row = np.ascontiguousarray(embeddings[input_id])
    maps = []
    for c in range(NC):
        lo, hi = c * HC, (c + 1) * HC
        gsel = np.concatenate(
            [w_ih[lo:hi], w_ih[H + lo:H + hi], w_ih[2 * H + lo:2 * H + hi],
             w_hh[lo:hi], w_hh[H + lo:H + hi], w_hh[2 * H + lo:2 * H + hi]],
            axis=0,
        )
        gb = np.concatenate(
            [b_ih[lo:hi], b_ih[H + lo:H + hi], b_ih[2 * H + lo:2 * H + hi],
             b_hh[lo:hi], b_hh[H + lo:H + hi], b_hh[2 * H + lo:2 * H + hi]],
        )
        v0 = c * VS
        nrows = min(VS, max(0, V - v0))
        wsh = np.zeros((VS, H), f)
        wsh[:nrows] = out_w[v0:v0 + nrows]
        wsh = wsh.astype(ml_dtypes.bfloat16)
        bsh = np.full((VS,), PAD_B, f)
        bsh[:nrows] = out_bv[v0:v0 + nrows]
        maps.append({
            "ones_in": np.ones((P,), f),
            "enc_sh": np.ascontiguousarray(enc[c * SC:(c + 1) * SC]),
            "emb": emb_row,
            "h0": hidden,
            "h0_sl": np.ascontiguousarray(hidden[lo:hi]),
            "comb_wT": np.ascontiguousarray(comb_w[lo:hi, :].T),
            "comb_b": np.ascontiguousarray(comb_b[lo:hi]),
            "gru_wT": np.ascontiguousarray(gsel.T),
            "gru_b": np.ascontiguousarray(gb),
            "out_wT": np.ascontiguousarray(wsh.T),
            "out_b": bsh,
        })
    return maps


def _assemble(results):
    lp = np.concatenate([results[c]["out_lp"] for c in range(NC)])[:V]
    log_probs = np.ascontiguousarray(lp.reshape(1, V))
    h_new = np.ascontiguousarray(results[0]["out_h"].reshape(1, 1, H))
    attn_weights = np.ones((1, S), np.float32)
    return log_probs, h_new, attn_weights


def _run(inputs, trace=False, trace_cores=None):
    import concourse.bass_utils as bass_utils

    nc = _get_compiled()
    maps = _prep(inputs)
    res = bass_utils.run_bass_kernel_spmd(
        nc, maps, core_ids=list(range(NC)), trace=trace, trace_cores=trace_cores,
    )
    return res


def kernel(**inputs):
    res = _run(inputs, trace=False)
    return _assemble(res.results)
